# revision 3
# baseline (speedup 1.0000x reference)
"""MixtureOfDepths router kernel for 8 Trainium2 NeuronCores.

Problem (hardcoded): hidden_states (4, 8192, 4096) f32, w (4096,) f32, b ()
  logits = hs @ w + b; weights = sigmoid(logits); k = 4096
  threshold = k-th largest weight per batch row; mask = weights >= threshold

Sharding: core c handles batch c//2, sequence half c%2 -> (4096, 4096) f32
slice (64 MiB).  The pair of cores exchanges computed router weights via
two small AllGathers (plus one tiny warmup AllGather that absorbs the
~50us ncfw cold-start off the critical path).

Stream: 16 two-slot tiles [128 x 8192] f32 alternating between the sync
and scalar HWDGE queues (~310 GB/s sustained; the gpsimd queue only runs
AllGather triggers / gather loads / wall broadcasts so SWDGE never blocks
descriptor generation).  Per slot (tiles 0-14): DVE tensor_tensor f32*f32
multiply with bf16 product output, ACT activation(Copy) accumulate
row-sum -> fp32 logit column.  The last tile's two slots use DVE STT fp32
dot-accumulate so the final sigmoid -> AllGather chain never waits on the
ACT row-sum queue.  ACT sigmoid per chunk -> weights (f32 out).

Top-k threshold, all arithmetic f32-exact (verified: DVE int32 adds are
NOT exact, so every base/candidate update stays in small-mantissa f32):
  1. 512-bin histogram (bin 2^21) of the f32 bit patterns of the FIRST
     chunk's weights (both cores; wall region 0 = 4096 values broadcast
     to all partitions), scanned in-stream: DVE counts candidate sets
     q=0,1 (is_ge + accumulate), ACT counts q=2,3 (Sign activation +
     accumulate; count = (N + sum sign)/2, boundary ties cost +-0.5
     which the refinement spans absorb).
  2. Resolve: flags vs K, row-sum, PE ones-matmul = sum+broadcast in one
     op -> base0 = (sumF-1)*2^21.  Counts miss the 4096 not-yet-gathered
     values, so base0 only under-shoots; round A's 128*2^19 = 2^26 span
     re-covers the worst-case rank drift (~2^25.6) with margin.
  3. Two refinement rounds over the full 8192-value wall (region 1 lands
     right after the stream ends; its AllGather runs post-stream where
     the collective turnaround is short): step 2^19 then step 2^12,
     each split DVE (region 0, is_ge) || ACT (region 1, Sign).
     threshold = final base; the 2^12-ulp bracket admits ~1 extra token
     per row past the exact k-th value.
Measured: ~300-313us device exec (max core, neuron-profile), weights max
rel err ~1.1e-3 (bf16 product rounding), mask ~14/32768 mismatches --
far inside the 2e-2 harness gate.  (Baseline streamed the same data but
ran fp32 STT dots on DVE only: ~400us device, 3.04ms reported.)
"""

import sys

if "/opt/trn_rl_repo" not in sys.path:
    sys.path.insert(0, "/opt/trn_rl_repo")

from contextlib import ExitStack

import numpy as np

import concourse.bass as bass  # noqa: F401
import concourse.tile as tile
from concourse import bacc, mybir
from concourse import bass2jax
from concourse import mybir as _mb

N_CORES = 8
BATCH = 4
SEQ = 8192
HIDDEN = 4096
TOK = SEQ // 2          # 4096 own tokens per core
K = SEQ // 2            # 4096 = top-k per batch row
NSLOT = 32              # own slots; token t = p*32 + s
TPS = 2                 # slots per stream tile
CHUNKS = [(0, 16), (16, 32)]   # AG chunk slot ranges
NSCAN = 1               # region 0 feeds the histogram; region 1 is
                        # only counted in the refinement rounds
NQ = 4                  # histogram candidate sets (512 bins total)
BIN = 1 << 21           # histogram bin width (f32 bit-int space)
STEP_A = 1 << 19        # refinement round A step (span 2^26)
STEP_B = 1 << 12        # refinement round B step (span 2^19)


def build(n_cores=N_CORES, pair_groups=None, fake_gather=False):
    f32, i32, u8 = mybir.dt.float32, mybir.dt.int32, mybir.dt.uint8
    bf16 = mybir.dt.bfloat16
    if pair_groups is None:
        pair_groups = [[2 * i, 2 * i + 1] for i in range(n_cores // 2)]

    nc = bacc.Bacc("TRN2", target_bir_lowering=False, debug=False,
                   num_devices=n_cores)

    hs = nc.dram_tensor("hs", [TOK, HIDDEN], f32, kind="ExternalInput").ap()
    w2 = nc.dram_tensor("w2", [128, HIDDEN], f32, kind="ExternalInput").ap()
    bias2 = nc.dram_tensor("bias2", [128, 1], f32, kind="ExternalInput").ap()
    # cfa: col0 = p*2^19, col1 = p*2^12 (f32)
    cfa = nc.dram_tensor("cfa", [128, 2], f32, kind="ExternalInput").ap()
    # cin: col q = (128q + p)*2^21 (i32 bit-int candidates), q = 0,1
    cin = nc.dram_tensor("cin", [128, 2], i32, kind="ExternalInput").ap()
    # cnegv: col j = -bitcast_f32((128(2+j) + p)*2^21)  (ACT Sign biases)
    cnegv = nc.dram_tensor("cnegv", [128, 2], f32, kind="ExternalInput").ap()
    wout = nc.dram_tensor("wout", [128, NSLOT], f32, kind="ExternalOutput").ap()
    mout = nc.dram_tensor("mout", [128, NSLOT], u8, kind="ExternalOutput").ap()

    hs2 = hs.rearrange("(p n s) d -> p n (s d)", p=128, s=TPS)

    chunk_cols = [hi - lo for lo, hi in CHUNKS]          # [16, 16]
    wall_widths = [2 * c * 128 for c in chunk_cols]      # [4096, 4096]
    wall_off = [sum(wall_widths[:j]) for j in range(2)]
    WALLW = sum(wall_widths)                             # 8192
    NPART = sum(wall_widths[:NSCAN])                     # 6144 partial values

    with tile.TileContext(nc) as tc, ExitStack() as ctx:
        consts = ctx.enter_context(tc.tile_pool(name="consts", bufs=1))
        hpool = ctx.enter_context(tc.tile_pool(name="hid", bufs=3))
        ppool = ctx.enter_context(tc.tile_pool(name="prod", bufs=2))
        wallp = ctx.enter_context(tc.tile_pool(name="wall", bufs=1))
        gsp = ctx.enter_context(tc.tile_pool(name="gs", bufs=1))
        small = ctx.enter_context(tc.tile_pool(name="small", bufs=1))
        psum = ctx.enter_context(tc.tile_pool(name="ps", bufs=2, space="PSUM"))
        dram = ctx.enter_context(tc.tile_pool(name="dram", bufs=1, space="DRAM"))

        # ---- consts on the scalar HWDGE queue ----
        wb = consts.tile([128, HIDDEN], f32)
        nc.scalar.dma_start(out=wb[:], in_=w2[:])
        bb = consts.tile([128, 1], f32)
        nc.scalar.dma_start(out=bb[:], in_=bias2[:])
        cfab = consts.tile([128, 2], f32)
        nc.scalar.dma_start(out=cfab[:], in_=cfa[:])
        cinb = consts.tile([128, 2], i32)
        nc.scalar.dma_start(out=cinb[:], in_=cin[:])
        cnegb = consts.tile([128, 2], f32)
        nc.scalar.dma_start(out=cnegb[:], in_=cnegv[:])
        ones128 = consts.tile([128, 128], f32)
        nc.vector.memset(ones128[:], 1.0)

        logits = small.tile([128, NSLOT], f32, tag="logits")
        wsig = small.tile([128, NSLOT], f32, tag="wsig")
        # DVE histogram counts: q=0,1 (region 0 only)
        GP = small.tile([128, 2], f32, tag="GP")
        # ACT sign-sums: q=2,3 (region 0 only)
        GA = small.tile([128, 2], f32, tag="GA")
        junk8 = small.tile([128, 4096], u8, tag="junk8")      # DVE scan dst
        ajunk = small.tile([128, HIDDEN], bf16, tag="ajunk")  # ACT copy dst
        sjunk = small.tile([128, 4096], u8, tag="sjunk")      # ACT sign dst

        wall = wallp.tile([128, WALLW], f32, tag="wall", name="wall")

        gins, gouts = [], []
        for j, c in enumerate(chunk_cols):
            gins.append(dram.tile([128, c], f32, name=f"gin{j}"))
            gouts.append(dram.tile([1, wall_widths[j]], f32, name=f"gout{j}"))

        sig = mybir.ActivationFunctionType.Sigmoid
        cp = mybir.ActivationFunctionType.Copy
        sgn = mybir.ActivationFunctionType.Sign

        def stream_tile(i, use_stt=False):
            ht = hpool.tile([128, TPS * HIDDEN], f32, tag="ht")
            dma_eng = nc.sync if i % 2 == 0 else nc.scalar
            dma_eng.dma_start(out=ht[:], in_=hs2[:, i, :])
            for s in range(TPS):
                slot = i * TPS + s
                hslice = ht[:, s * HIDDEN:(s + 1) * HIDDEN]
                if use_stt:
                    sj = ppool.tile([128, HIDDEN], bf16, tag="prod")
                    nc.vector.scalar_tensor_tensor(
                        out=sj[:], in0=hslice, scalar=1.0, in1=wb[:],
                        op0=mybir.AluOpType.mult, op1=mybir.AluOpType.mult,
                        accum_out=logits[:, slot:slot + 1])
                else:
                    prod = ppool.tile([128, HIDDEN], bf16, tag="prod")
                    nc.vector.tensor_tensor(
                        out=prod[:], in0=hslice, in1=wb[:],
                        op=mybir.AluOpType.mult)
                    nc.scalar.activation(
                        out=ajunk[:], in_=prod[:], func=cp,
                        accum_out=logits[:, slot:slot + 1])

        def chunk_post(j):
            lo, hi = CHUNKS[j]
            nc.scalar.activation(out=wsig[:, lo:hi], in_=logits[:, lo:hi],
                                 func=sig, bias=bb[:])
            nc.scalar.dma_start(out=wout[:, lo:hi], in_=wsig[:, lo:hi])
            nc.gpsimd.dma_start(out=gins[j][:], in_=wsig[:, lo:hi])
            if fake_gather:
                g2 = gouts[j][:].rearrange("a (h t) -> a h t", h=2)
                nc.scalar.dma_start(out=g2[:, 0, :], in_=gins[j].opt())
                nc.scalar.dma_start(out=g2[:, 1, :], in_=gins[j].opt())
            else:
                nc.gpsimd.collective_compute(
                    "AllGather", mybir.AluOpType.bypass,
                    replica_groups=pair_groups,
                    ins=[gins[j].opt()], outs=[gouts[j].opt()])
            gs = gsp.tile([1, wall_widths[j]], f32, tag="gsrow")
            nc.gpsimd.dma_start(out=gs[:], in_=gouts[j][:])
            nc.gpsimd.partition_broadcast(
                wall[:, wall_off[j]:wall_off[j] + wall_widths[j]], gs[:],
                channels=128)

        def dve_scan(j, q):
            # GP[p, q] = #{x in region 0 : x >= bitcast((128q+p)*2^21)}
            lo, w = wall_off[j], wall_widths[j]
            nc.vector.tensor_scalar(
                out=junk8[:, 0:w], in0=wall[:, lo:lo + w],
                scalar1=cinb[:, q:q + 1].bitcast(f32), scalar2=None,
                op0=mybir.AluOpType.is_ge, op1=mybir.AluOpType.add,
                accum_out=GP[:, q:q + 1])

        def act_scan(j, jq):
            # GA[p, jq] = sum sign(x - value(cand_{q=2+jq})) in region 0
            lo, w = wall_off[j], wall_widths[j]
            nc.scalar.activation(
                out=sjunk[:, 0:w], in_=wall[:, lo:lo + w],
                func=sgn, bias=cnegb[:, jq:jq + 1],
                accum_out=GA[:, jq:jq + 1])

        # ---- warmup collective: absorbs ncfw cold-start off-path ----
        gwin = dram.tile([128, 1], f32, name="gwin")
        gwout = dram.tile([1, 256], f32, name="gwout")
        nc.gpsimd.dma_start(out=gwin[:], in_=bias2[:])
        nc.gpsimd.collective_compute(
            "AllGather", mybir.AluOpType.bypass,
            replica_groups=pair_groups,
            ins=[gwin.opt()], outs=[gwout.opt()])

        # ---- the stream ----
        for i in range(8):                      # T0-T7: slots 0-15
            stream_tile(i)
        chunk_post(0)
        for i in range(8, 13):                  # T8-T12: slots 16-25
            stream_tile(i)
        stream_tile(13)
        dve_scan(0, 0)
        act_scan(0, 0)
        stream_tile(14)
        dve_scan(0, 1)
        act_scan(0, 1)
        stream_tile(15, use_stt=True)           # T15: slots 30-31
        chunk_post(1)

        # ---- resolve partial histogram (region 0; 4096 values) ----
        C = small.tile([128, NQ], f32, tag="C")
        nc.vector.tensor_copy(C[:, 0:2], GP[:])
        nc.vector.tensor_scalar(
            out=C[:, 2:4], in0=GA[:], scalar1=float(NPART), scalar2=0.5,
            op0=mybir.AluOpType.add, op1=mybir.AluOpType.mult)
        F = small.tile([128, NQ], f32, tag="F")
        nc.vector.tensor_scalar(
            out=F[:], in0=C[:], scalar1=float(K), scalar2=None,
            op0=mybir.AluOpType.is_ge)
        f1 = small.tile([128, 1], f32, tag="f1")
        nc.vector.tensor_scalar(
            out=junk8[:, 0:NQ], in0=F[:], scalar1=0.0, scalar2=0.0,
            op0=mybir.AluOpType.add, op1=mybir.AluOpType.add,
            accum_out=f1[:])
        ps0 = psum.tile([128, 1], f32, tag="psb")
        nc.tensor.matmul(ps0[:], lhsT=ones128[:], rhs=f1[:],
                         start=True, stop=True)
        # base0 = (sumF - 1) * 2^21   (f32-exact)
        base0 = small.tile([128, 1], f32, tag="base0")
        nc.vector.tensor_scalar(
            out=base0[:], in0=ps0[:], scalar1=1.0, scalar2=float(BIN),
            op0=mybir.AluOpType.subtract, op1=mybir.AluOpType.mult)

        def refine(base, step, cf_col, tag):
            # one radix round over the FULL wall: DVE counts region 0,
            # ACT Sign-counts region 1; returns new base (f32-exact)
            cand = small.tile([128, 1], i32, tag=f"cand{tag}")
            nc.vector.tensor_tensor(out=cand[:], in0=base[:],
                                    in1=cfab[:, cf_col:cf_col + 1],
                                    op=mybir.AluOpType.add)
            neg = small.tile([128, 1], f32, tag=f"neg{tag}")
            nc.vector.tensor_scalar(
                out=neg[:], in0=cand[:].bitcast(f32), scalar1=-1.0,
                scalar2=None, op0=mybir.AluOpType.mult)
            c1 = small.tile([128, 1], f32, tag=f"c1{tag}")
            nc.vector.tensor_scalar(
                out=junk8[:, 0:4096], in0=wall[:, 0:4096],
                scalar1=cand[:].bitcast(f32), scalar2=None,
                op0=mybir.AluOpType.is_ge, op1=mybir.AluOpType.add,
                accum_out=c1[:])
            s2 = small.tile([128, 1], f32, tag=f"s2{tag}")
            nc.scalar.activation(
                out=sjunk[:, 0:4096], in_=wall[:, 4096:WALLW],
                func=sgn, bias=neg[:], accum_out=s2[:])
            c2 = small.tile([128, 1], f32, tag=f"c2{tag}")
            nc.vector.tensor_scalar(
                out=c2[:], in0=s2[:], scalar1=float(wall_widths[1]),
                scalar2=0.5, op0=mybir.AluOpType.add,
                op1=mybir.AluOpType.mult)
            cnt = small.tile([128, 1], f32, tag=f"cnt{tag}")
            nc.vector.tensor_tensor(out=cnt[:], in0=c1[:], in1=c2[:],
                                    op=mybir.AluOpType.add)
            flag = small.tile([128, 1], f32, tag=f"flag{tag}")
            nc.vector.tensor_scalar(
                out=flag[:], in0=cnt[:], scalar1=float(K), scalar2=None,
                op0=mybir.AluOpType.is_ge)
            ps = psum.tile([128, 1], f32, tag="psb")
            nc.tensor.matmul(ps[:], lhsT=ones128[:], rhs=flag[:],
                             start=True, stop=True)
            d = small.tile([128, 1], f32, tag=f"d{tag}")
            nc.vector.tensor_scalar(
                out=d[:], in0=ps[:], scalar1=1.0, scalar2=float(step),
                op0=mybir.AluOpType.subtract, op1=mybir.AluOpType.mult)
            nb = small.tile([128, 1], f32, tag=f"nb{tag}")
            nc.vector.tensor_tensor(out=nb[:], in0=base[:], in1=d[:],
                                    op=mybir.AluOpType.add)
            return nb

        baseA = refine(base0, STEP_A, 0, "A")
        baseB = refine(baseA, STEP_B, 1, "B")
        ti = small.tile([128, 1], i32, tag="ti")
        nc.vector.tensor_copy(ti[:], baseB[:])

        # ---- mask: own weights >= threshold ----
        mask = small.tile([128, NSLOT], u8, tag="mask")
        nc.vector.tensor_scalar(
            out=mask[:], in0=wsig[:], scalar1=ti[:].bitcast(f32),
            scalar2=None, op0=mybir.AluOpType.is_ge)
        nc.sync.dma_start(out=mout[:], in_=mask[:])

    nc.compile()
    return nc


class Runner:
    """Executes a built Bass module on the 8 axon NeuronCores via PJRT."""

    def __init__(self, nc, n_cores=N_CORES):
        import jax
        from jax.sharding import Mesh, PartitionSpec
        from jax.experimental.shard_map import shard_map

        bass2jax.install_neuronx_cc_hook()
        self.n_cores = n_cores
        partition_name = (nc.partition_id_tensor.name
                          if nc.partition_id_tensor else None)
        in_names, out_names, out_avals, zero_outs = [], [], [], []
        for alloc in nc.m.functions[0].allocations:
            if not isinstance(alloc, _mb.MemoryLocationSet):
                continue
            name = alloc.memorylocations[0].name
            if alloc.kind == "ExternalInput":
                if name != partition_name:
                    in_names.append(name)
            elif alloc.kind == "ExternalOutput":
                shape = tuple(alloc.tensor_shape)
                dtype = _mb.dt.np(alloc.dtype)
                out_names.append(name)
                out_avals.append(jax.core.ShapedArray(shape, dtype))
                zero_outs.append(np.zeros(shape, dtype))
        self.in_names, self.out_names = list(in_names), out_names
        self.out_avals, self.zero_outs = out_avals, zero_outs
        n_params, n_outs = len(in_names), len(out_avals)
        self.n_params = n_params
        all_names = in_names + out_names
        if partition_name is not None:
            all_names = all_names + [partition_name]

        def _body(*args):
            operands = list(args)
            if partition_name is not None:
                operands.append(bass2jax.partition_id_tensor())
            return tuple(bass2jax._bass_exec_p.bind(
                *operands,
                out_avals=tuple(out_avals),
                in_names=tuple(all_names),
                out_names=tuple(out_names),
                lowering_input_output_aliases=(),
                sim_require_finite=True,
                sim_require_nnan=True,
                nc=nc,
            ))

        devices = jax.devices()[:n_cores]
        self.mesh = Mesh(np.asarray(devices), ("core",))
        self.pspec = PartitionSpec("core")
        in_specs = (self.pspec,) * (n_params + n_outs)
        out_specs = (self.pspec,) * n_outs
        self.sharded = jax.jit(
            shard_map(_body, mesh=self.mesh, in_specs=in_specs,
                      out_specs=out_specs, check_rep=False),
            donate_argnums=tuple(range(n_params, n_params + n_outs)),
            keep_unused=True)

    def concat_inputs(self, in_maps):
        return [np.concatenate([np.asarray(in_maps[c][nm])
                                for c in range(self.n_cores)], axis=0)
                for nm in self.in_names]

    def fresh_zeros(self):
        return [np.zeros((self.n_cores * z.shape[0], *z.shape[1:]), z.dtype)
                for z in self.zero_outs]

    def call(self, concat_in):
        return self.sharded(*concat_in, *self.fresh_zeros())

    def run(self, in_maps):
        out_arrs = self.call(self.concat_inputs(in_maps))
        return [
            {nm: np.asarray(out_arrs[i]).reshape(
                self.n_cores, *self.out_avals[i].shape)[c]
             for i, nm in enumerate(self.out_names)}
            for c in range(self.n_cores)
        ]


_NC_CACHE = {}


def _get_nc():
    if "full" not in _NC_CACHE:
        _NC_CACHE["full"] = build()
    return _NC_CACHE["full"]


def _get_runner():
    if "runner" not in _NC_CACHE:
        _NC_CACHE["runner"] = Runner(_get_nc())
    return _NC_CACHE["runner"]


def make_in_maps(hidden_states, w, b, n_cores=N_CORES, tok=TOK):
    hs = np.asarray(hidden_states, dtype=np.float32)
    wv = np.asarray(w, dtype=np.float32).reshape(-1)
    hidden = wv.shape[0]
    w2 = np.ascontiguousarray(np.broadcast_to(wv[None, :], (128, hidden)))
    bias2 = np.full((128, 1), np.float32(b), dtype=np.float32)
    p = np.arange(128, dtype=np.int64)
    cfa = np.stack([(p << 19), (p << 12)], axis=1).astype(np.float32)
    cin = np.stack([(128 * q + p) << 21 for q in range(2)],
                   axis=1).astype(np.int32)
    cnegv = -np.stack(
        [((128 * (q + 2) + p) << 21).astype(np.int32).view(np.float32)
         for q in range(2)], axis=1).astype(np.float32)
    in_maps = []
    for c in range(n_cores):
        bb, h = c // 2, c % 2
        own = hs[bb, h * tok:(h + 1) * tok, :]
        in_maps.append({"hs": np.ascontiguousarray(own), "w2": w2,
                        "bias2": bias2, "cfa": cfa, "cin": cin,
                        "cnegv": cnegv})
    return in_maps


def assemble(results, n_cores=N_CORES, tok=TOK):
    weights = np.empty((BATCH, SEQ), dtype=np.float32)
    mask = np.empty((BATCH, SEQ), dtype=bool)
    for c in range(n_cores):
        bb, h = c // 2, c % 2
        weights[bb, h * tok:(h + 1) * tok] = results[c]["wout"].reshape(-1)
        mask[bb, h * tok:(h + 1) * tok] = results[c]["mout"].reshape(-1) != 0
    return weights, mask


def kernel(hidden_states, w, b):
    runner = _get_runner()
    in_maps = make_in_maps(hidden_states, w, b)
    return assemble(runner.run(in_maps))
